# revision 1
# baseline (speedup 1.0000x reference)
"""Trainium2 Bass kernel for BaselineProtonet (retrieval_knn).

logits[q, c] = -||query_q - proto_c||_2
  proto_c = mean of 64 support embeddings of class c
  embeddings_stacked: [64 classes * (64 support + 64 query), 1024] f32

Sharding (8 cores): query-sharded, support-replicated. Core i owns query
rows 512i..512(i+1); every core receives the full support set (fp8 on
the wire) and computes all 64 prototypes locally on the TensorEngine, so
no cross-core collective is needed (a ncfw collective costs ~50us of
control latency in this runtime, far more than the extra DMA).

Host-side shard prep (layout/encoding only, no arithmetic): support is
pre-swizzled to the exact SBUF layout (contiguous per-partition runs so
HWDGE descriptor generation is cheap) and encoded fp8e4m3; queries are
transposed to feature-major (d on partitions) and encoded bf16.

Per core:
  protos   : 32 fp8 DoubleRow one-hot matmuls (256 support rows each)
             accumulate class sums -> PSUM [64,1024] f32, scaled 1/64
             on ACT evacuation -> bf16 prototypes
  P^T      : 8 PE transposes -> W = -2*P^T (bf16, ACT scale)
  ||p||^2  : ACT square-accumulate on prototypes -> [64,1] f32, added
             per-partition (class) via the ACT sqrt bias
  ||q||^2  : DVE squares + all-ones-stationary matmuls accumulated
             straight into the Gram PSUM group (broadcasts sum_d q_d^2
             to every class row); these open the group and track the
             query stream while the W chain completes
  Gram     : 8 accumulating matmuls lhsT=W chunk, rhs=Q^T chunk (bf16)
  logits   : -sqrt(dist^2) via ACT sqrt(+bias) and DVE negate, in two
             pipelined query halves; output [64, 512] (class-major);
             the host transposes/concats the per-core blocks.
PE is pre-warmed with dummy matmuls during the DMA wait (HAM clock gate)
and the sqrt ACT table is preloaded by a dummy activation.
"""

import numpy as np

C = 64          # classes
S = 64          # support per class (== queries per class)
D = 1024        # embedding dim
NCORES = 8
CL = C // NCORES            # 8 classes per core's query shard
QL = CL * S                 # 512 query rows per core
DCH = D // 128              # 8 d-chunks
SCH = (C * S) // 128        # 32 support row chunks (full support)

_CACHE = {}


def _emit(nc, tc, sup, qt, oh_in, out):
    """Emit the per-core tile program.

    sup:   [128, SCH*D] fp8 DRAM  (full support, swizzled: row p of
                                   chunk j holds support row j*128+p)
    qt:    [128, DCH*QL] bf16 DRAM (queries, swizzled feature-major)
    oh_in: [128, SCH*C] fp8 DRAM  (DoubleRow one-hot class masks)
    out:   [C, QL] f32 DRAM       (negated distances, class-major)
    """
    from concourse import masks, mybir

    f32 = mybir.dt.float32
    bf16 = mybir.dt.bfloat16
    fp8 = mybir.dt.float8e4
    AF = mybir.ActivationFunctionType

    with (
        tc.tile_pool(name="sb", bufs=1) as sb,
        tc.tile_pool(name="ps", bufs=1, space="PSUM") as ps,
    ):
        # warm the PE clock first-thing (HAM gate needs ~3.5us of busy
        # before the real matmuls; deps are a single DVE memset)
        wm_in = sb.tile([128, 512], bf16)
        nc.vector.memset(wm_in[:], 0.0)
        wm_ps = ps.tile([128, 512], f32)
        for _ in range(7):
            nc.tensor.matmul(
                wm_ps[:], wm_in[:, 0:128], wm_in[:], start=True, stop=True
            )

        # ---------------- input DMAs (one sync-ring FIFO: one-hot, the
        # support stream, then query quarters -- slices drain in order;
        # the prototype matmuls track the support stream and the
        # Gram/norm matmuls track the query stream)
        oh = sb.tile([128, SCH // 2, 2, C], fp8)
        nc.scalar.dma_start(
            oh[:], oh_in[:, :].rearrange("p (j o k) -> p j o k", j=SCH // 2, o=2)
        )
        s8 = sb.tile([128, SCH, D], fp8)
        for b in range(8):
            nc.sync.dma_start(
                s8[:, 4 * b : 4 * (b + 1)],
                sup[:, 4 * b * D : 4 * (b + 1) * D].rearrange(
                    "p (c d) -> p c d", c=4
                ),
            )
        q16 = sb.tile([128, DCH, QL], bf16)
        for h in range(4):
            nc.sync.dma_start(
                q16[:, 2 * h : 2 * (h + 1)],
                qt[:, 2 * h * QL : 2 * (h + 1) * QL].rearrange(
                    "p (k q) -> p k q", k=2
                ),
            )

        # ---------------- constants -------------------------------------
        ident = sb.tile([128, 128], bf16)
        masks.make_identity(nc, ident[:])
        ones64 = sb.tile([128, C], bf16)
        nc.gpsimd.memset(ones64[:], 1.0)

        # preload the sqrt ACT table set off the critical path
        warm_sq = sb.tile([1, 1], f32)
        nc.gpsimd.memset(warm_sq[:], 1.0)
        nc.scalar.activation(warm_sq[:], warm_sq[:], AF.Sqrt)

        # ---------------- prototypes (all 64 classes) -------------------
        # fp8 DoubleRow: each matmul contracts 256 support rows (chunk
        # pair jp), streaming 2 rows/cycle through the PE array
        s8v = s8[:].rearrange("p (jp o) d -> p jp o d", o=2)
        p_ps = ps.tile([C, D], f32)  # [64, 1024] = 2 banks
        for jp in range(SCH // 2):
            for h in range(2):
                nc.tensor.matmul(
                    p_ps[:, 512 * h : 512 * (h + 1)],
                    oh[:, jp],
                    s8v[:, jp, :, 512 * h : 512 * (h + 1)],
                    start=(jp == 0),
                    stop=(jp == SCH // 2 - 1),
                    perf_mode=mybir.MatmulPerfMode.DoubleRow,
                )
        # ---------------- ||q||^2 squares (per chunk, DVE) ---------------
        qsq = sb.tile([128, DCH, QL], bf16)
        for k in range(DCH):
            nc.vector.tensor_mul(qsq[:, k], q16[:, k], q16[:, k])

        # ||q||^2 matmuls open the s_ps PSUM group and track the query
        # stream while the W chain (evac -> transpose -> scale) completes
        # on ACT/PE; the Gram matmuls are appended after W below.
        # s_ps[c, q] = sum_k ( ones^T qsq_k + W_k^T q_k ) = ||q||^2 - 2 q.p
        s_ps = ps.tile([C, QL], f32)
        for k in range(DCH):
            nc.tensor.matmul(
                s_ps[:], ones64[:], qsq[:, k], start=(k == 0), stop=False
            )

        # evacuate the two d-halves on ACT (DVE is busy with the query
        # squares; separate tiles so Tile doesn't serialize the writers)
        psbA = sb.tile([C, 512], bf16)
        psbB = sb.tile([C, 512], bf16)
        nc.scalar.mul(psbA[:], p_ps[:, 0:512], 1.0 / S)
        nc.scalar.mul(psbB[:], p_ps[:, 512:1024], 1.0 / S)

        # ---------------- W = -2 * P^T (bf16, ACT evac) ------------------
        pt_ps = ps.tile([128, DCH * C], bf16)  # chunk k at cols 64k..64k+64
        W = sb.tile([128, DCH, C], bf16)
        for k in range(DCH):
            half = psbA if k < 4 else psbB
            nc.tensor.transpose(
                pt_ps[:, C * k : C * (k + 1)],
                half[:, 128 * (k % 4) : 128 * (k % 4 + 1)],
                ident[0:C, 0:C],
            )
        nc.scalar.mul(W[:], pt_ps[:], -2.0)

        # ||p||^2 in f32 via ACT square-accumulate (consistent with the
        # bf16 protos used in the Gram); halves summed on DVE
        pn_dump = sb.tile([C, D], bf16)
        pnA = sb.tile([C, 1], f32)
        pnB = sb.tile([C, 1], f32)
        pn_col = sb.tile([C, 1], f32)
        nc.scalar.activation(pn_dump[:, 0:512], psbA[:], AF.Square, accum_out=pnA[:])
        nc.scalar.activation(
            pn_dump[:, 512:1024], psbB[:], AF.Square, accum_out=pnB[:]
        )

        # ------- Gram matmuls (follow the W chain) -----------------------
        for k in range(DCH):
            nc.tensor.matmul(
                s_ps[:], W[:, k], q16[:, k], start=False, stop=(k == DCH - 1)
            )

        nc.vector.tensor_add(pn_col[:], pnA[:], pnB[:])

        # ------- sqrt(+||p||^2), negate, store (2 q-halves pipelined) ----
        lt = sb.tile([C, QL], f32)
        for hq in range(2):
            s = slice(256 * hq, 256 * (hq + 1))
            nc.scalar.activation(lt[:, s], s_ps[:, s], AF.Sqrt, bias=pn_col[:, 0:1])
            nc.vector.tensor_scalar_mul(lt[:, s], lt[:, s], -1.0)
            nc.scalar.dma_start(out[:, s], lt[:, s])


def _build():
    if "nc" in _CACHE:
        return _CACHE["nc"]
    from concourse import bacc, mybir, tile

    f32 = mybir.dt.float32
    bf16 = mybir.dt.bfloat16
    fp8 = mybir.dt.float8e4
    nc = bacc.Bacc(
        "TRN2",
        target_bir_lowering=False,
        debug=False,
        enable_asserts=False,
        num_devices=NCORES,
    )
    sup = nc.dram_tensor("sup", [128, SCH * D], fp8, kind="ExternalInput").ap()
    qt = nc.dram_tensor("qt", [128, DCH * QL], bf16, kind="ExternalInput").ap()
    oh_in = nc.dram_tensor("oh", [128, SCH * C], fp8, kind="ExternalInput").ap()
    out = nc.dram_tensor("out", [C, QL], f32, kind="ExternalOutput").ap()
    with tile.TileContext(nc) as tc:
        _emit(nc, tc, sup, qt, oh_in, out)
    nc.compile()
    _CACHE["nc"] = nc
    return nc


def _onehot():
    import ml_dtypes

    # DoubleRow one-hot: oh[p, jp, o, c] = 1 iff class c owns support row
    # (2*jp + o)*128 + p, i.e. c == 4*jp + 2*o + p//64
    p = np.arange(128)[:, None, None, None]
    jp = np.arange(SCH // 2)[None, :, None, None]
    o = np.arange(2)[None, None, :, None]
    c = np.arange(C)[None, None, None, :]
    oh = (c == 4 * jp + 2 * o + p // 64).astype(ml_dtypes.float8_e4m3)
    return np.ascontiguousarray(oh.reshape(128, SCH * C))


def _shard(embeddings):
    import ml_dtypes

    emb = np.asarray(embeddings, dtype=np.float32).reshape(C, 2 * S, D)
    # support: [C*S, D] -> swizzled [128, SCH, D] (row p of chunk j =
    # support row j*128+p), fp8 on the wire
    sup = emb[:, :S, :].reshape(SCH, 128, D).transpose(1, 0, 2)
    sup = np.ascontiguousarray(
        sup.astype(ml_dtypes.float8_e4m3).reshape(128, SCH * D)
    )
    oh = _onehot()
    in_maps = []
    for i in range(NCORES):
        q = emb[CL * i : CL * (i + 1), S:, :].reshape(QL, D)
        # Q^T [D, QL] -> swizzled [128, DCH, QL] bf16
        qt_i = q.T.reshape(DCH, 128, QL).transpose(1, 0, 2)
        qt_i = np.ascontiguousarray(
            qt_i.astype(ml_dtypes.bfloat16).reshape(128, DCH * QL)
        )
        in_maps.append({"sup": sup, "qt": qt_i, "oh": oh})
    return in_maps


def kernel(embeddings_stacked, n_classes, n_support, **_unused):
    assert int(n_classes) == C and int(n_support) == S
    emb = np.asarray(embeddings_stacked)
    assert emb.shape == (C * 2 * S, D), emb.shape

    from concourse import bass_utils

    nc = _build()
    in_maps = _shard(emb)
    try:
        res = bass_utils.run_bass_kernel_spmd(
            nc, in_maps, core_ids=list(range(NCORES))
        )
    except Exception:
        # transient device/runtime hiccups have been observed; retry once
        res = bass_utils.run_bass_kernel_spmd(
            nc, in_maps, core_ids=list(range(NCORES))
        )
    logits = np.empty((C * S, C), dtype=np.float32)
    for i in range(NCORES):
        logits[QL * i : QL * (i + 1), :] = res.results[i]["out"].T
    return logits


if __name__ == "__main__":
    rng = np.random.default_rng(0)
    emb = rng.standard_normal((C * 2 * S, D), dtype=np.float32)
    got = kernel(emb, C, S)
    print("kernel output", got.shape, got.dtype)



# revision 2
# speedup vs baseline: 2.5176x; 2.5176x over previous
"""Trainium2 Bass kernel for BaselineProtonet (retrieval_knn).

logits[q, c] = -||query_q - proto_c||_2
  proto_c = mean of 64 support embeddings of class c
  embeddings_stacked: [64 classes * (64 support + 64 query), 1024] f32

Sharding (8 cores): 2D-balanced grid, 4 query-groups x 2 class-halves.
Core (a, b) owns query rows 1024a..1024(a+1) and classes 32b..32b+32, so
it reads 1MB of queries + 2MB of support (both fp8 on the wire) instead
of the 4.5MB a pure query shard would need -- the input DMA is the
dominant cost and this is the byte-optimal integer grid. No cross-core
collective (a ncfw collective costs ~65us of control latency in this
runtime, measured).

Host-side shard prep (layout/encoding only, no arithmetic): support is
pre-swizzled to contiguous per-partition runs and encoded fp8e4m3;
queries are transposed feature-major (d on partitions) and encoded fp8
(the kernel uses the rounded values consistently in both the Gram and
||q||^2 terms, so fp8 queries shift each distance by a sub-tolerance
amount rather than decorrelating the terms).

Per core:
  protos   : 8 fp8 DoubleRow one-hot matmuls (256 support rows each)
             accumulate class sums -> PSUM [32,1024] f32
  ||q||^2  : DVE squares + a 2-level pairwise chunk reduction (8->4->2
             chunks, bf16), then 4 all-ones-stationary matmuls open the
             Gram PSUM group (broadcast sum_d q_d^2 to every class row)
  P^T      : ACT evac (scale 1/64 -> bf16), 8 PE transposes, ACT scale
             -2 -> W fp8 [128, 8, 32]
  ||p||^2  : ACT square-accumulate on the bf16 prototypes -> [32,1] f32
  Gram     : 8 fp8 DoubleRow matmuls lhsT=W pair, rhs=Q^T pair, emitted
             N-half-major so the first output half closes early
  logits   : -sqrt(dist^2) via ACT sqrt(+||p||^2 bias) and DVE negate in
             four pipelined quarters; output [32, 1024] (class-major);
             the host transposes the per-core blocks into place.
Input DMAs are split across both HWDGE rings (sync + scalar) so issue
overlaps; PE is pre-warmed with dummy matmuls (HAM clock gate) and all
ACT tables (Copy/Square/Sqrt) are preloaded by dummy activations.
"""

import numpy as np

C = 64          # classes
S = 64          # support per class (== queries per class)
D = 1024        # embedding dim
NCORES = 8
QA = 4          # query groups
CB = 2          # class halves
CL = C // CB    # 32 classes per core
NQ = (C * S) // QA          # 1024 query rows per core
DCH = D // 128              # 8 d-chunks
SCH = (CL * S) // 128       # 16 support row chunks per core

_CACHE = {}


def _emit(nc, tc, sup, qt, oh_in, out):
    """Emit the per-core tile program.

    sup:   [128, SCH*D] fp8 DRAM  (support half, swizzled: row p of
                                   chunk j holds shard support row j*128+p)
    qt:    [128, DCH*NQ] fp8 DRAM (queries, swizzled feature-major)
    oh_in: [128, (SCH//2)*2*CL] fp8 DRAM (DoubleRow one-hot class masks)
    out:   [CL, NQ] f32 DRAM      (negated distances, class-major)
    """
    from concourse import masks, mybir

    f32 = mybir.dt.float32
    bf16 = mybir.dt.bfloat16
    fp8 = mybir.dt.float8e4
    AF = mybir.ActivationFunctionType

    with (
        tc.tile_pool(name="sb", bufs=1) as sb,
        tc.tile_pool(name="ps", bufs=1, space="PSUM") as ps,
    ):
        # warm the PE clock first-thing (HAM gate needs ~3.5us of busy
        # before the real matmuls; deps are a single DVE memset)
        wm_in = sb.tile([128, 512], bf16)
        nc.vector.memset(wm_in[:], 0.0)
        wm_ps = ps.tile([128, 512], f32)
        for _ in range(7):
            nc.tensor.matmul(
                wm_ps[:], wm_in[:, 0:128], wm_in[:], start=True, stop=True
            )

        # ---------------- input DMAs, split across both HWDGE rings.
        # scalar ring: support chunks 0-7 (nothing queued ahead, so the
        # early proto pairs land first); sync ring: queries then support
        # chunks 8-15. Total 3MB; the SDMA pool drains both rings.
        s8 = sb.tile([128, SCH, D], fp8)
        for b in range(2):
            nc.scalar.dma_start(
                s8[:, 4 * b : 4 * (b + 1)],
                sup[:, 4 * b * D : 4 * (b + 1) * D].rearrange(
                    "p (c d) -> p c d", c=4
                ),
            )
        q8 = sb.tile([128, DCH, NQ], fp8)
        for h in range(2):
            nc.sync.dma_start(
                q8[:, 4 * h : 4 * (h + 1)],
                qt[:, 4 * h * NQ : 4 * (h + 1) * NQ].rearrange(
                    "p (k q) -> p k q", k=4
                ),
            )
        for b in range(2, 4):
            nc.sync.dma_start(
                s8[:, 4 * b : 4 * (b + 1)],
                sup[:, 4 * b * D : 4 * (b + 1) * D].rearrange(
                    "p (c d) -> p c d", c=4
                ),
            )
        oh = sb.tile([128, SCH // 2, 2, CL], fp8)
        nc.gpsimd.dma_start(
            oh[:], oh_in[:, :].rearrange("p (j o k) -> p j o k", j=SCH // 2, o=2)
        )

        # ---------------- constants -------------------------------------
        ident = sb.tile([128, 128], bf16)
        masks.make_identity(nc, ident[:])
        ones = sb.tile([128, CL], bf16)
        nc.gpsimd.memset(ones[:], 1.0)

        # preload every ACT table set off the critical path (Copy for the
        # evacs, Square for ||p||^2, Sqrt for the distances)
        warm_t = sb.tile([1, 1], f32)
        warm_d = sb.tile([1, 1], bf16)
        warm_a = sb.tile([1, 1], f32)
        nc.gpsimd.memset(warm_t[:], 1.0)
        nc.scalar.mul(warm_d[:], warm_t[:], 1.0)
        nc.scalar.activation(warm_d[:], warm_t[:], AF.Square, accum_out=warm_a[:])
        nc.scalar.activation(warm_t[:], warm_t[:], AF.Sqrt)

        # ---------------- ||q||^2 squares + 2-level reduce (DVE) ---------
        qsq = sb.tile([128, DCH, NQ], bf16)
        for k in range(DCH):
            nc.vector.tensor_mul(qsq[:, k], q8[:, k], q8[:, k])
        qp4 = sb.tile([128, 4, NQ], bf16)
        for m in range(4):
            nc.vector.tensor_add(qp4[:, m], qsq[:, 2 * m], qsq[:, 2 * m + 1])
        qsqs = sb.tile([128, 2, NQ], bf16)
        for h2 in range(2):
            nc.vector.tensor_add(qsqs[:, h2], qp4[:, 2 * h2], qp4[:, 2 * h2 + 1])

        # ---------------- prototypes (this core's 32 classes) -----------
        # fp8 DoubleRow: each matmul contracts 256 support rows (chunk
        # pair jp), streaming 2 rows/cycle through the PE array
        s8v = s8[:].rearrange("p (jp o) d -> p jp o d", o=2)
        p_ps = ps.tile([CL, D], f32)
        for jp in range(SCH // 2):
            for h in range(2):
                nc.tensor.matmul(
                    p_ps[:, 512 * h : 512 * (h + 1)],
                    oh[:, jp],
                    s8v[:, jp, :, 512 * h : 512 * (h + 1)],
                    start=(jp == 0),
                    stop=(jp == SCH // 2 - 1),
                    perf_mode=mybir.MatmulPerfMode.DoubleRow,
                )

        # ||q||^2 matmuls open the s_ps PSUM group; they fill the PE gap
        # between the last proto matmul and the first transpose.
        # s_ps[c, q] = sum_h ones^T qsqs_h + sum_kp W_kp^T q_kp
        #            = ||q||^2 - 2 q.p
        s_ps = ps.tile([CL, NQ], f32)
        for hq in range(2):
            for n in range(2):
                nc.tensor.matmul(
                    s_ps[:, 512 * n : 512 * (n + 1)],
                    ones[:],
                    qsqs[:, hq, 512 * n : 512 * (n + 1)],
                    start=(hq == 0),
                    stop=False,
                )

        # evacuate the two d-halves on ACT (prototypes = sums * 1/S)
        psbA = sb.tile([CL, 512], bf16)
        psbB = sb.tile([CL, 512], bf16)
        nc.scalar.mul(psbA[:], p_ps[:, 0:512], 1.0 / S)
        nc.scalar.mul(psbB[:], p_ps[:, 512:1024], 1.0 / S)

        # ---------------- W = -2 * P^T (fp8, ACT evac) -------------------
        pt_ps = ps.tile([128, DCH, CL], bf16)
        W = sb.tile([128, DCH, CL], fp8)
        for k in range(DCH):
            half = psbA if k < 4 else psbB
            nc.tensor.transpose(
                pt_ps[:, k],
                half[:, 128 * (k % 4) : 128 * (k % 4 + 1)],
                ident[0:CL, 0:CL],
            )
        nc.scalar.mul(W[:, 0:4], pt_ps[:, 0:4], -2.0)
        nc.scalar.mul(W[:, 4:8], pt_ps[:, 4:8], -2.0)

        # ||p||^2 in f32 via ACT square-accumulate on the bf16 protos
        pn_dump = sb.tile([CL, D], bf16)
        pnA = sb.tile([CL, 1], f32)
        pnB = sb.tile([CL, 1], f32)
        pn_col = sb.tile([CL, 1], f32)
        nc.scalar.activation(pn_dump[:, 0:512], psbA[:], AF.Square, accum_out=pnA[:])
        nc.scalar.activation(
            pn_dump[:, 512:1024], psbB[:], AF.Square, accum_out=pnB[:]
        )
        nc.vector.tensor_add(pn_col[:], pnA[:], pnB[:])

        # ------- Gram matmuls (fp8 DoubleRow), N-half-major so the first
        # output half closes while the second is still streaming ---------
        q8v = q8[:].rearrange("p (kp o) q -> p kp o q", o=2)
        for n in range(2):
            for kp in range(DCH // 2):
                nc.tensor.matmul(
                    s_ps[:, 512 * n : 512 * (n + 1)],
                    W[:, 2 * kp : 2 * kp + 2],
                    q8v[:, kp, :, 512 * n : 512 * (n + 1)],
                    start=False,
                    stop=(kp == DCH // 2 - 1),
                    perf_mode=mybir.MatmulPerfMode.DoubleRow,
                )

        # ------- sqrt(+||p||^2), negate, store (4 quarters pipelined) ----
        lt = sb.tile([CL, NQ], f32)
        for qi in range(4):
            s = slice(256 * qi, 256 * (qi + 1))
            nc.scalar.activation(
                lt[:, s], s_ps[:, s], AF.Sqrt, bias=pn_col[:, 0:1]
            )
            nc.vector.tensor_scalar_mul(lt[:, s], lt[:, s], -1.0)
        for hn in range(2):
            s = slice(512 * hn, 512 * (hn + 1))
            nc.sync.dma_start(out[:, s], lt[:, s])


def _build():
    if "nc" in _CACHE:
        return _CACHE["nc"]
    from concourse import bacc, mybir, tile

    f32 = mybir.dt.float32
    fp8 = mybir.dt.float8e4
    nc = bacc.Bacc(
        "TRN2",
        target_bir_lowering=False,
        debug=False,
        enable_asserts=False,
        num_devices=NCORES,
    )
    sup = nc.dram_tensor("sup", [128, SCH * D], fp8, kind="ExternalInput").ap()
    qt = nc.dram_tensor("qt", [128, DCH * NQ], fp8, kind="ExternalInput").ap()
    oh_in = nc.dram_tensor(
        "oh", [128, (SCH // 2) * 2 * CL], fp8, kind="ExternalInput"
    ).ap()
    out = nc.dram_tensor("out", [CL, NQ], f32, kind="ExternalOutput").ap()
    with tile.TileContext(nc) as tc:
        _emit(nc, tc, sup, qt, oh_in, out)
    nc.compile()
    _CACHE["nc"] = nc
    return nc


def _onehot():
    import ml_dtypes

    # DoubleRow one-hot: oh[p, jp, o, c] = 1 iff (shard-local) class c
    # owns shard support row (2*jp + o)*128 + p, i.e. c == 4*jp + 2*o + p//64
    p = np.arange(128)[:, None, None, None]
    jp = np.arange(SCH // 2)[None, :, None, None]
    o = np.arange(2)[None, None, :, None]
    c = np.arange(CL)[None, None, None, :]
    oh = (c == 4 * jp + 2 * o + p // 64).astype(ml_dtypes.float8_e4m3)
    return np.ascontiguousarray(oh.reshape(128, (SCH // 2) * 2 * CL))


def _shard(embeddings):
    import ml_dtypes

    emb = np.asarray(embeddings, dtype=np.float32).reshape(C, 2 * S, D)
    # support halves: classes 32b..32b+32 -> shard rows r = c_local*64+s,
    # swizzled [128, SCH, D] (row p of chunk j = shard row j*128+p), fp8
    sups = []
    for b in range(CB):
        shard = emb[CL * b : CL * (b + 1), :S, :].reshape(CL * S, D)
        sw = shard.reshape(SCH, 128, D).transpose(1, 0, 2)
        sups.append(
            np.ascontiguousarray(
                sw.astype(ml_dtypes.float8_e4m3).reshape(128, SCH * D)
            )
        )
    # query groups: rows 1024a..1024(a+1) of the query set, feature-major
    qry = emb[:, S:, :].reshape(C * S, D)
    qts = []
    for a in range(QA):
        q = qry[NQ * a : NQ * (a + 1)]
        qt_i = q.T.reshape(DCH, 128, NQ).transpose(1, 0, 2)
        qts.append(
            np.ascontiguousarray(
                qt_i.astype(ml_dtypes.float8_e4m3).reshape(128, DCH * NQ)
            )
        )
    oh = _onehot()
    in_maps = []
    for i in range(NCORES):
        a, b = i // CB, i % CB
        in_maps.append({"sup": sups[b], "qt": qts[a], "oh": oh})
    return in_maps


def _assemble(outs):
    """outs: per-core [CL, NQ] blocks -> full [C*S, C] logits."""
    logits = np.empty((C * S, C), dtype=np.float32)
    for i in range(NCORES):
        a, b = i // CB, i % CB
        logits[NQ * a : NQ * (a + 1), CL * b : CL * (b + 1)] = outs[i].T
    return logits


def kernel(embeddings_stacked, n_classes, n_support, **_unused):
    assert int(n_classes) == C and int(n_support) == S
    emb = np.asarray(embeddings_stacked)
    assert emb.shape == (C * 2 * S, D), emb.shape

    from concourse import bass_utils

    nc = _build()
    in_maps = _shard(emb)
    try:
        res = bass_utils.run_bass_kernel_spmd(
            nc, in_maps, core_ids=list(range(NCORES))
        )
    except Exception:
        # transient device/runtime hiccups have been observed; retry once
        res = bass_utils.run_bass_kernel_spmd(
            nc, in_maps, core_ids=list(range(NCORES))
        )
    return _assemble([res.results[i]["out"] for i in range(NCORES)])


if __name__ == "__main__":
    rng = np.random.default_rng(0)
    emb = rng.standard_normal((C * 2 * S, D), dtype=np.float32)
    got = kernel(emb, C, S)
    print("kernel output", got.shape, got.dtype)
